# revision 1
# baseline (speedup 1.0000x reference)
"""Trainium2 Bass kernel for nn_ReallocationMapEncoder.

The reference network is three NAC layers (y = x @ (tanh(W_hat)*sigmoid(M_hat)).T)
applied to a [nsteps, nsyms, nsyms, 3] grid of normalized (t, a, b) indices,
plus a gb broadcast on the trailing axis. NAC is linear in x, so the whole
network collapses to one effective matrix Weff = W3 @ W2 @ W1 of shape [2, 3]:

    y[t, a, b, c] = gb[c] + (t/2)*Weff[c,0] + (a/2048)*Weff[c,1] + (b/2048)*Weff[c,2]

The output [2, 2048, 2048, 2] f32 (67 MB) is a separable affine ramp; the kernel
is purely output-write-bandwidth bound (memory regime).

Device strategy (8 cores, data-parallel on the `a` axis, 256 rows each, so
each core writes 8.4 MB): one DVE iota builds J[p, j] = j once; every output
slice [128 a-rows, 2048 b-cols at c-stride] is then a single fused DVE
tensor_scalar

    out[p, b, c] = J[p, b] * (Weff[c,2]/nsyms) + bias[p, (t,blk,c)]

where bias (a tiny [128, 8] per-core input) folds the gb/t/a terms:
bias[p, t,blk,c] = gb[c] + (t/2)*Weff[c,0] + (a(p,blk)/2048)*Weff[c,1].

Sync-wait slot limits in walrus codegen (HWDGE DMA: 1, DVE/ACT: 2) shape the
structure: single compute engine, one out-DMA per [128, 4096] tile, and at
most 8 total DMAs so DMAHW queues are never reused.
"""

import numpy as np

NSTEPS = 2
NSYMS = 2048
NCORES = 8
A_PER_CORE = NSYMS // NCORES          # 256
BLKS = A_PER_CORE // 128              # 2 partition blocks per core
F = NSYMS * 2                         # 4096 free elements per a-row (b, c interleaved)

_CACHE = {}


def _build_bass(scales):
    import concourse.bass as bass
    import concourse.mybir as mybir
    from concourse.tile import TileContext

    f32 = mybir.dt.float32
    nc = bass.Bass(trn_type="TRN2")

    bias_in = nc.dram_tensor("bias_in", [128, NSTEPS * BLKS * 2], f32, kind="ExternalInput")
    out = nc.dram_tensor("out", [NSTEPS, BLKS, 128, F], f32, kind="ExternalOutput")

    with TileContext(nc) as tc:
        with (
            tc.tile_pool(name="const", bufs=1) as const,
            tc.tile_pool(name="outp", bufs=4) as outp,
        ):
            bias_sb = const.tile([128, NSTEPS * BLKS * 2], f32)
            nc.gpsimd.dma_start(bias_sb[:], bias_in[:])

            J = const.tile([128, NSYMS], f32)
            nc.gpsimd.iota(
                J[:], pattern=[[1, NSYMS]], base=0, channel_multiplier=0,
                allow_small_or_imprecise_dtypes=True,
            )

            # This walrus build fits exactly ONE semaphore wait per
            # instruction. Two tiny observer copies make DVE's vector clock
            # see the iota (Pool sem) and the bias DMA (DMAHW sem) one at a
            # time, so the real tensor_scalar ops below need no waits at
            # all, and with bufs=4 no output slot is ever reused.
            scratch = const.tile([1, 2], f32)
            nc.vector.tensor_copy(scratch[:, 0:1], J[0:1, 0:1])
            nc.vector.tensor_copy(scratch[:, 1:2], bias_sb[0:1, 0:1])

            for t in range(NSTEPS):
                for blk in range(BLKS):
                    ot = outp.tile([128, F], f32)
                    otv = ot[:].rearrange("p (b c) -> p b c", c=2)
                    for c in range(2):
                        idx = (t * BLKS + blk) * 2 + c
                        nc.vector.tensor_scalar(
                            otv[:, :, c],
                            J[:],
                            scales[c],
                            bias_sb[:, idx : idx + 1],
                            mybir.AluOpType.mult,
                            mybir.AluOpType.add,
                        )
                    nc.gpsimd.dma_start(out[t, blk], ot[:])

    _legalize_waits(nc, mybir)
    return nc


def _legalize_waits(nc, mybir):
    """This walrus build fits very few semaphore waits per instruction (one
    for most engine structs). Tile's auto-generated kernel-tail drain waits
    on every DMA lane + engine sem at once; split any multi-wait instruction
    into a chain of single-wait Drain carriers on the same engine."""
    for func in nc.m.functions:
        for block in func.blocks:
            insts = list(block.instructions)
            new_insts = []
            changed = False
            for inst in insts:
                si = inst.sync_info
                waits = list(si.on_wait) if si is not None and si.on_wait else []
                if len(waits) > 1:
                    for w in waits[:-1]:
                        d = mybir.InstDrain(
                            name=f"{inst.name}-waitsplit-{len(new_insts)}",
                            ins=[],
                            outs=[],
                            bass_is_fusable=False,
                        )
                        d.engine = inst.engine
                        d.sync_info = mybir.SyncInfo(on_wait=[w], on_update=[])
                        new_insts.append(d)
                    inst.sync_info = mybir.SyncInfo(
                        on_wait=[waits[-1]], on_update=list(si.on_update or [])
                    )
                    changed = True
                new_insts.append(inst)
            if changed:
                block.instructions = new_insts


def _host_consts(gb, w_hat1, m_hat1, w_hat2, m_hat2, w_hat3, m_hat3):
    def nacw(w, m):
        w = np.asarray(w, np.float64)
        m = np.asarray(m, np.float64)
        return np.tanh(w) * (1.0 / (1.0 + np.exp(-m)))

    weff = nacw(w_hat3, m_hat3) @ nacw(w_hat2, m_hat2) @ nacw(w_hat1, m_hat1)  # [2,3]
    gb = np.asarray(gb, np.float64)

    scales = [float(np.float32(weff[c, 2] / NSYMS)) for c in range(2)]

    # bias[core][p, (t,blk,c)] = gb[c] + (t/2)Weff[c,0] + (a/2048)Weff[c,1]
    biases = []
    for core in range(NCORES):
        bias = np.empty((128, NSTEPS, BLKS, 2), np.float64)
        for t in range(NSTEPS):
            for blk in range(BLKS):
                a = (core * A_PER_CORE + blk * 128 + np.arange(128)) / NSYMS
                for c in range(2):
                    bias[:, t, blk, c] = (
                        gb[c] + (t / NSTEPS) * weff[c, 0] + a * weff[c, 1]
                    )
        biases.append(np.ascontiguousarray(bias.reshape(128, -1), np.float32))
    return scales, biases


def kernel(market, gb, w_hat1, m_hat1, w_hat2, m_hat2, w_hat3, m_hat3):
    from concourse.bass_utils import run_bass_kernel_spmd

    scales, biases = _host_consts(gb, w_hat1, m_hat1, w_hat2, m_hat2, w_hat3, m_hat3)
    # the tensor_scalar immediates (scales) are baked into the traced program,
    # so the compiled module is keyed on them
    key = ("nc", tuple(scales))
    if key not in _CACHE:
        _CACHE[key] = _build_bass(scales)
    nc = _CACHE[key]
    _CACHE["last_nc"] = nc

    in_maps = [{"bias_in": biases[core]} for core in range(NCORES)]
    res = run_bass_kernel_spmd(nc, in_maps, core_ids=list(range(NCORES)))
    parts = [r["out"].reshape(NSTEPS, A_PER_CORE, NSYMS, 2) for r in res.results]
    return np.concatenate(parts, axis=1)



# revision 3
# speedup vs baseline: 1.0590x; 1.0590x over previous
"""Trainium2 Bass kernel for nn_ReallocationMapEncoder.

The reference network is three NAC layers (y = x @ (tanh(W_hat)*sigmoid(M_hat)).T)
applied to a [nsteps, nsyms, nsyms, 3] grid of normalized (t, a, b) indices,
plus a gb broadcast on the trailing axis. NAC is linear in x, so the whole
network collapses to one effective matrix Weff = W3 @ W2 @ W1 of shape [2, 3]:

    y[t, a, b, c] = gb[c] + (t/2)*Weff[c,0] + (a/2048)*Weff[c,1] + (b/2048)*Weff[c,2]

The output [2, 2048, 2048, 2] f32 (67 MB) is a separable affine ramp; the kernel
is purely output-write-bandwidth bound (memory regime).

Device strategy (8 cores, data-parallel on `a`, 256 rows = 2 partition blocks
per core, so each core writes 8.4 MB as four [128, 4096] (b,c)-interleaved
tiles): out[p, 2b+c] = J[b]*scale_c + bias[p, (t,blk,c)].

v2 (HWDGE + dual-engine) — the v1 profile showed SWDGE Q7 descriptor
emission (~5 us per 2 MB dma_start) serialized the whole drain and the
single-engine compute delayed the first transfer to ~16 us:
  * all DMAs issued on the two HWDGE rings (nc.sync=SP, nc.scalar=ACT):
    RTL descriptor generation, ~0.6 us fixed cost, no Q7 in the loop.
    Measured SDMA drain rate is ~411 GB/s, so 8.4 MB ~ 20.4 us.
  * compute split DVE/ACT (DVE ~0.62 ns/el, ACT ~0.83 ns/el, imbalanced
    shares: DVE 10240 els/partition, ACT 6144) so tiles are ready at the
    drain rate; ACT-computed tiles are DMA'd by ACT itself (program order,
    zero semaphore waits), DVE tiles by SP (one DVE-sem wait each).
  * iota J in 3 pieces (512/512/1024) on Pool so the first chunks of every
    tile can start as soon as the first 512 b-columns exist.
  * every DMA reads a chunk-tile that is fully written before the DMA and
    never written again -> exactly <=1 wait per DMA under any dependency
    granularity; walrus wait-slot limits (HWDGE: 1, DVE/ACT: 2) hold.
Output chunking (per (t,blk) tile of 4096 free cols = 2048 b x 2 c):
  tile0 (t0,b0, DVE): cols [0:1024) + [1024:4096)     -> SP DMAs
  tile1 (t0,b1):      cols [0:1024) ACT + [1024:2048) ACT + [2048:4096) DVE/SP
  tile2 (t1,b0, DVE): whole                            -> SP
  tile3 (t1,b1, ACT): whole                            -> ACT
8 DMAs total (bias-in + 7 out) = the 8 DMAHW semaphore lanes, none reused.
"""

import numpy as np

NSTEPS = 2
NSYMS = 2048
NCORES = 8
A_PER_CORE = NSYMS // NCORES          # 256
BLKS = A_PER_CORE // 128              # 2 partition blocks per core
F = NSYMS * 2                         # 4096 free elements per a-row (b, c interleaved)

_CACHE = {}


def _build_bass(scales):
    import concourse.bass as bass
    import concourse.mybir as mybir
    from concourse.tile import TileContext

    f32 = mybir.dt.float32
    nc = bass.Bass(trn_type="TRN2")

    bias_in = nc.dram_tensor("bias_in", [128, NSTEPS * BLKS * 2], f32, kind="ExternalInput")
    out = nc.dram_tensor("out", [NSTEPS, BLKS, 128, F], f32, kind="ExternalOutput")

    Ident = mybir.ActivationFunctionType.Identity

    with TileContext(nc) as tc:
        with (
            tc.tile_pool(name="const", bufs=1) as const,
            tc.tile_pool(name="outp", bufs=1) as outp,
        ):
            bias_sb = const.tile([128, NSTEPS * BLKS * 2], f32)
            # HWDGE input DMA on the SP ring; no Q7, no waits.
            nc.sync.dma_start(bias_sb[:], bias_in[:])

            # J ramp in 3 pieces so early chunks unblock fast.
            J0 = const.tile([128, 512], f32)
            J1 = const.tile([128, 512], f32)
            J2 = const.tile([128, 1024], f32)
            for jt, base, n in ((J0, 0, 512), (J1, 512, 512), (J2, 1024, 1024)):
                nc.gpsimd.iota(
                    jt[:], pattern=[[1, n]], base=base, channel_multiplier=0,
                    allow_small_or_imprecise_dtypes=True,
                )

            def bcol(t, blk, c):
                idx = (t * BLKS + blk) * 2 + c
                return bias_sb[:, idx : idx + 1]

            def dve_op(view, jt, t, blk, c):
                nc.vector.tensor_scalar(
                    view, jt[:], scales[c], bcol(t, blk, c),
                    mybir.AluOpType.mult, mybir.AluOpType.add,
                )

            def act_op(view, jt, t, blk, c):
                nc.scalar.activation(
                    view, jt[:], Ident, bias=bcol(t, blk, c), scale=scales[c],
                )

            # chunk-tiles (widths in interleaved cols; /2 = b-range width)
            ot0a = outp.tile([128, 1024], f32)   # t0,b0 cols [0:1024)      DVE
            ot0b = outp.tile([128, 3072], f32)   # t0,b0 cols [1024:4096)   DVE
            ot2 = outp.tile([128, F], f32)       # t1,b0 whole              DVE
            ot1r = outp.tile([128, 2048], f32)   # t0,b1 cols [2048:4096)   DVE
            ot1a = outp.tile([128, 1024], f32)   # t0,b1 cols [0:1024)      ACT
            ot1m = outp.tile([128, 1024], f32)   # t0,b1 cols [1024:2048)   ACT
            ot3 = outp.tile([128, F], f32)       # t1,b1 whole              ACT

            def iv(tile):
                return tile[:].rearrange("p (b c) -> p b c", c=2)

            # ---- DVE stream (tiles t0b0, t1b0, + R-chunk of t0b1) ----
            for c in range(2):
                dve_op(iv(ot0a)[:, :, c], J0, 0, 0, c)            # ops 1-2
            for c in range(2):
                dve_op(iv(ot2)[:, 0:512, c], J0, 1, 0, c)         # ops 3-4
            for c in range(2):
                dve_op(iv(ot0b)[:, 0:512, c], J1, 0, 0, c)        # ops 5-6
            for c in range(2):
                dve_op(iv(ot2)[:, 512:1024, c], J1, 1, 0, c)      # ops 7-8
            for c in range(2):
                dve_op(iv(ot0b)[:, 512:1536, c], J2, 0, 0, c)     # ops 9-10
            for c in range(2):
                dve_op(iv(ot1r)[:, :, c], J2, 0, 1, c)            # ops 11-12
            for c in range(2):
                dve_op(iv(ot2)[:, 1024:2048, c], J2, 1, 0, c)     # ops 13-14

            # ---- ACT stream (tiles t0b1 A+M, t1b1 whole) + its own DMAs ----
            for c in range(2):
                act_op(iv(ot1a)[:, :, c], J0, 0, 1, c)
            nc.scalar.dma_start(out[0, 1, :, 0:1024], ot1a[:])
            for c in range(2):
                act_op(iv(ot3)[:, 0:512, c], J0, 1, 1, c)
            for c in range(2):
                act_op(iv(ot1m)[:, :, c], J1, 0, 1, c)
            nc.scalar.dma_start(out[0, 1, :, 1024:2048], ot1m[:])
            for c in range(2):
                act_op(iv(ot3)[:, 512:1024, c], J1, 1, 1, c)
            for c in range(2):
                act_op(iv(ot3)[:, 1024:2048, c], J2, 1, 1, c)
            nc.scalar.dma_start(out[1, 1], ot3[:])

            # ---- SP ring: DMAs for the DVE-computed chunks (1 wait each) ----
            nc.sync.dma_start(out[0, 0, :, 0:1024], ot0a[:])      # after op 2
            nc.sync.dma_start(out[0, 0, :, 1024:4096], ot0b[:])   # after op 10
            nc.sync.dma_start(out[0, 1, :, 2048:4096], ot1r[:])   # after op 12
            nc.sync.dma_start(out[1, 0], ot2[:])                  # after op 14

    _legalize_waits(nc, mybir)
    return nc


def _legalize_waits(nc, mybir):
    """Walrus codegen allows very few semaphore waits per instruction (one
    for most engine structs). Tile's auto-generated kernel-tail drain waits
    on every DMA lane + engine sem at once; split any multi-wait instruction
    into a chain of single-wait Drain carriers on the same engine."""
    for func in nc.m.functions:
        for block in func.blocks:
            insts = list(block.instructions)
            new_insts = []
            changed = False
            for inst in insts:
                si = inst.sync_info
                waits = list(si.on_wait) if si is not None and si.on_wait else []
                if len(waits) > 1:
                    for w in waits[:-1]:
                        d = mybir.InstDrain(
                            name=f"{inst.name}-waitsplit-{len(new_insts)}",
                            ins=[],
                            outs=[],
                            bass_is_fusable=False,
                        )
                        d.engine = inst.engine
                        d.sync_info = mybir.SyncInfo(on_wait=[w], on_update=[])
                        new_insts.append(d)
                    inst.sync_info = mybir.SyncInfo(
                        on_wait=[waits[-1]], on_update=list(si.on_update or [])
                    )
                    changed = True
                new_insts.append(inst)
            if changed:
                block.instructions = new_insts


def _host_consts(gb, w_hat1, m_hat1, w_hat2, m_hat2, w_hat3, m_hat3):
    def nacw(w, m):
        w = np.asarray(w, np.float64)
        m = np.asarray(m, np.float64)
        return np.tanh(w) * (1.0 / (1.0 + np.exp(-m)))

    weff = nacw(w_hat3, m_hat3) @ nacw(w_hat2, m_hat2) @ nacw(w_hat1, m_hat1)  # [2,3]
    gb = np.asarray(gb, np.float64)

    scales = [float(np.float32(weff[c, 2] / NSYMS)) for c in range(2)]

    # bias[core][p, (t,blk,c)] = gb[c] + (t/2)Weff[c,0] + (a/2048)Weff[c,1]
    biases = []
    for core in range(NCORES):
        bias = np.empty((128, NSTEPS, BLKS, 2), np.float64)
        for t in range(NSTEPS):
            for blk in range(BLKS):
                a = (core * A_PER_CORE + blk * 128 + np.arange(128)) / NSYMS
                for c in range(2):
                    bias[:, t, blk, c] = (
                        gb[c] + (t / NSTEPS) * weff[c, 0] + a * weff[c, 1]
                    )
        biases.append(np.ascontiguousarray(bias.reshape(128, -1), np.float32))
    return scales, biases


def kernel(market, gb, w_hat1, m_hat1, w_hat2, m_hat2, w_hat3, m_hat3):
    from concourse.bass_utils import run_bass_kernel_spmd

    scales, biases = _host_consts(gb, w_hat1, m_hat1, w_hat2, m_hat2, w_hat3, m_hat3)
    # the tensor_scalar immediates (scales) are baked into the traced program,
    # so the compiled module is keyed on them
    key = ("nc", tuple(scales))
    if key not in _CACHE:
        _CACHE[key] = _build_bass(scales)
    nc = _CACHE[key]
    _CACHE["last_nc"] = nc

    in_maps = [{"bias_in": biases[core]} for core in range(NCORES)]
    res = run_bass_kernel_spmd(nc, in_maps, core_ids=list(range(NCORES)))
    parts = [r["out"].reshape(NSTEPS, A_PER_CORE, NSYMS, 2) for r in res.results]
    return np.concatenate(parts, axis=1)


# revision 7
# speedup vs baseline: 1.1506x; 1.0865x over previous
"""Trainium2 Bass kernel for nn_ReallocationMapEncoder.

The reference network is three NAC layers (y = x @ (tanh(W_hat)*sigmoid(M_hat)).T)
applied to a [nsteps, nsyms, nsyms, 3] grid of normalized (t, a, b) indices,
plus a gb broadcast on the trailing axis. NAC is linear in x, so the whole
network collapses to one effective matrix Weff = W3 @ W2 @ W1 of shape [2, 3]:

    y[t, a, b, c] = gb[c] + (t/2)*Weff[c,0] + (a/2048)*Weff[c,1] + (b/2048)*Weff[c,2]

The output [2, 2048, 2048, 2] f32 (67 MB) is a separable affine ramp; the kernel
is purely output-write-bandwidth bound (memory regime).

Device strategy (8 cores, data-parallel on `a`, 256 rows = 2 partition blocks
per core, so each core writes 8.4 MB as four [128, 4096] (b,c)-interleaved
tiles): out[p, 2b+c] = J[b]*scale_c + bias[p, (t,blk,c)].

v3 — lessons from the v1/v2 profiles:
  * v1: SWDGE Q7 descriptor emission (~5 us per 2 MB dma_start) paced the
    whole drain at ~400 GB/s and delayed first data to ~16 us.  44.7 us.
  * v2: two concurrent HWDGE rings (SP+ACT) made every SDMA engine
    round-robin between two queues at packet granularity, inflating
    per-16KB-packet time from ~610 ns to ~1010 ns (26.9 -> ~16 GB/s per
    engine; one engine finished 4 us after the rest).  42.3 us.
  So: ONE HWDGE ring (SP) for every DMA -> each of the 16 SDMA engines
  stays on one queue at ~26.9 GB/s = ~425 GB/s aggregate.
  * All compute on DVE (~0.53 ns/el + ~200 ns/op; ~975 GB/s of output
    bytes, 2.3x the drain rate, so a single engine keeps the ring fed).
    Two dependency-free warmup ops absorb the ~2 us first-op penalty
    while the bias DMA (4 KB, ~2.7 us dispatch->receipt) is in flight.
  * Output written as a ramp of chunks (128/384/512/1024 b-cols then
    1 MB pieces) so the first transfer is in flight ~1 us after the bias
    lands and the SDMA queue never starves afterwards.  SP's FIFO ring
    is ordered exactly by DVE completion order (every DMA: one wait on
    the DVE sem at an increasing threshold).
  * iota J in 3 pieces (512/512/1024) on Pool; chunk boundaries align
    with the pieces so every tensor_scalar reads a single J tile.
  * 12 DMAs on 8 DMAHW lanes: lanes reused only by late DMAs whose
    lane-predecessors (the early small chunks) completed long before;
    _legalize_waits splits any >1-wait instruction into Drain carriers.
"""

import numpy as np

NSTEPS = 2
NSYMS = 2048
NCORES = 8
A_PER_CORE = NSYMS // NCORES          # 256
BLKS = A_PER_CORE // 128              # 2 partition blocks per core
F = NSYMS * 2                         # 4096 free elements per a-row (b, c interleaved)

# per-(t,blk) tile chunking in b-columns; boundaries stay inside one J piece
_CHUNKS = {
    (0, 0): [(0, 128), (128, 512), (512, 1024), (1024, 2048)],
    (0, 1): [(0, 512), (512, 1024), (1024, 2048)],
    (1, 0): [(0, 1024), (1024, 2048)],
    (1, 1): [(0, 1024), (1024, 2048)],
}
_JPIECES = [(0, 512), (512, 1024), (1024, 2048)]

_CACHE = {}


def _jparts(b0, b1):
    """Intersections of [b0,b1) with the J pieces: (piece_idx, jlo, jhi, b_start)."""
    parts = []
    for idx, (j0, j1) in enumerate(_JPIECES):
        lo, hi = max(b0, j0), min(b1, j1)
        if lo < hi:
            parts.append((idx, lo - j0, hi - j0, lo))
    assert sum(hi - lo for _, lo, hi, _ in parts) == b1 - b0, (b0, b1)
    return parts


def _build_bass(scales):
    import concourse.bass as bass
    import concourse.mybir as mybir
    from concourse.tile import TileContext

    f32 = mybir.dt.float32
    nc = bass.Bass(trn_type="TRN2")

    bias_in = nc.dram_tensor("bias_in", [128, NSTEPS * BLKS * 2], f32, kind="ExternalInput")
    out = nc.dram_tensor("out", [NSTEPS, BLKS, 128, F], f32, kind="ExternalOutput")

    with TileContext(nc) as tc:
        with (
            tc.tile_pool(name="const", bufs=1) as const,
            tc.tile_pool(name="outp", bufs=1) as outp,
        ):
            bias_sb = const.tile([128, NSTEPS * BLKS * 2], f32)
            # single HWDGE ring (SP) for everything; first instr, no waits
            nc.sync.dma_start(bias_sb[:], bias_in[:])

            # DVE warmups: no deps, absorb the ~2us first-op penalty while
            # the bias DMA is in flight. Write/read a private scratch tile.
            scratch = const.tile([128, 4], f32)
            nc.vector.tensor_scalar(
                scratch[:, 0:1], scratch[:, 2:3], 1.0, None, mybir.AluOpType.mult
            )
            nc.vector.tensor_scalar(
                scratch[:, 1:2], scratch[:, 2:3], 1.0, None, mybir.AluOpType.mult
            )

            J = [
                const.tile([128, j1 - j0], f32, name=f"J{i}")
                for i, (j0, j1) in enumerate(_JPIECES)
            ]
            for jt, (j0, j1) in zip(J, _JPIECES):
                nc.gpsimd.iota(
                    jt[:], pattern=[[1, j1 - j0]], base=j0, channel_multiplier=0,
                    allow_small_or_imprecise_dtypes=True,
                )

            def bcol(t, blk, c):
                idx = (t * BLKS + blk) * 2 + c
                return bias_sb[:, idx : idx + 1]

            # chunk tiles, compute ops (all DVE), and the SP FIFO of DMAs
            # ordered exactly by completion.
            for t, blk in ((0, 0), (0, 1), (1, 0), (1, 1)):
                for b0, b1 in _CHUNKS[(t, blk)]:
                    ct = outp.tile([128, (b1 - b0) * 2], f32, name=f"ct_{t}_{blk}_{b0}")
                    cv = ct[:].rearrange("p (b c) -> p b c", c=2)
                    for jidx, o0, o1, bs in _jparts(b0, b1):
                        for c in range(2):
                            nc.vector.tensor_scalar(
                                cv[:, bs - b0 : bs - b0 + (o1 - o0), c],
                                J[jidx][:, o0:o1], scales[c],
                                bcol(t, blk, c),
                                mybir.AluOpType.mult, mybir.AluOpType.add,
                            )
                    nc.sync.dma_start(out[t, blk, :, b0 * 2 : b1 * 2], ct[:])

    _legalize_waits(nc, mybir)
    return nc


def _legalize_waits(nc, mybir):
    """Walrus codegen allows very few semaphore waits per instruction (one
    for most engine structs). Tile's auto-generated kernel-tail drain waits
    on every DMA lane + engine sem at once; split any multi-wait instruction
    into a chain of single-wait Drain carriers on the same engine."""
    for func in nc.m.functions:
        for block in func.blocks:
            insts = list(block.instructions)
            new_insts = []
            changed = False
            for inst in insts:
                si = inst.sync_info
                waits = list(si.on_wait) if si is not None and si.on_wait else []
                if len(waits) > 1:
                    for w in waits[:-1]:
                        d = mybir.InstDrain(
                            name=f"{inst.name}-waitsplit-{len(new_insts)}",
                            ins=[],
                            outs=[],
                            bass_is_fusable=False,
                        )
                        d.engine = inst.engine
                        d.sync_info = mybir.SyncInfo(on_wait=[w], on_update=[])
                        new_insts.append(d)
                    inst.sync_info = mybir.SyncInfo(
                        on_wait=[waits[-1]], on_update=list(si.on_update or [])
                    )
                    changed = True
                new_insts.append(inst)
            if changed:
                block.instructions = new_insts


def _host_consts(gb, w_hat1, m_hat1, w_hat2, m_hat2, w_hat3, m_hat3):
    def nacw(w, m):
        w = np.asarray(w, np.float64)
        m = np.asarray(m, np.float64)
        return np.tanh(w) * (1.0 / (1.0 + np.exp(-m)))

    weff = nacw(w_hat3, m_hat3) @ nacw(w_hat2, m_hat2) @ nacw(w_hat1, m_hat1)  # [2,3]
    gb = np.asarray(gb, np.float64)

    scales = [float(np.float32(weff[c, 2] / NSYMS)) for c in range(2)]

    # bias[core][p, (t,blk,c)] = gb[c] + (t/2)Weff[c,0] + (a/2048)Weff[c,1]
    biases = []
    for core in range(NCORES):
        bias = np.empty((128, NSTEPS, BLKS, 2), np.float64)
        for t in range(NSTEPS):
            for blk in range(BLKS):
                a = (core * A_PER_CORE + blk * 128 + np.arange(128)) / NSYMS
                for c in range(2):
                    bias[:, t, blk, c] = (
                        gb[c] + (t / NSTEPS) * weff[c, 0] + a * weff[c, 1]
                    )
        biases.append(np.ascontiguousarray(bias.reshape(128, -1), np.float32))
    return scales, biases


def kernel(market, gb, w_hat1, m_hat1, w_hat2, m_hat2, w_hat3, m_hat3):
    from concourse.bass_utils import run_bass_kernel_spmd

    scales, biases = _host_consts(gb, w_hat1, m_hat1, w_hat2, m_hat2, w_hat3, m_hat3)
    # the tensor_scalar immediates (scales) are baked into the traced program,
    # so the compiled module is keyed on them
    key = ("nc", tuple(scales))
    if key not in _CACHE:
        _CACHE[key] = _build_bass(scales)
    nc = _CACHE[key]
    _CACHE["last_nc"] = nc

    in_maps = [{"bias_in": biases[core]} for core in range(NCORES)]
    res = run_bass_kernel_spmd(nc, in_maps, core_ids=list(range(NCORES)))
    parts = [r["out"].reshape(NSTEPS, A_PER_CORE, NSYMS, 2) for r in res.results]
    return np.concatenate(parts, axis=1)
